# revision 1
# baseline (speedup 1.0000x reference)
"""Trainium2 Bass kernel for nn_CrossLayer (protein/drug cross-attention).

Reference math (per batch n):
  pg = group_mean(protein, 4)   # (512, 512)
  dg = group_mean(drug, 2)      # (128, 512)
  q/k/v projections (8 heads, dh=64), cross logits, softmax over the
  "other" sequence, attention-weighted values, masked mean-pool over the
  own sequence, concat(prot_embed, drug_embed) -> (1024,)

Key algebraic simplification used on device: the pooled output
  prot_embed_h = sum_l w[l] * (alpha_h @ vd_h)[l, :]
              = (u_h.T @ P_h) @ vd_h          with u_h = w / rowsum_h
so the full (L x L') attention-output einsum is never materialized —
only per-head vectors c_h = P_h.T @ u_h and a tiny c_h.T @ v matvec.

Sharding: data-parallel over batch N=64 across 8 cores (8 batches/core),
weights replicated. All matmuls in bf16 (PE full rate), fp32 PSUM accum.
"""

import sys

import numpy as np

for _p in ("/opt/trn_rl_repo", "/root/.axon_site/_ro/trn_rl_repo"):
    if _p not in sys.path:
        sys.path.insert(0, _p)

HID = 512
H = 8
DH = 64
GS_P = 4
GS_D = 2
LP_FULL = 2048
LD_FULL = 256
LP = LP_FULL // GS_P  # 512 grouped protein length
LD = LD_FULL // GS_D  # 128 grouped drug length
NB = 64  # total batch
NCORES = 8
B = NB // NCORES  # 8 batches per core
P = 128  # partitions

_CACHE = {}


def _numpy_reference(protein, drug, mask_prot, mask_drug, Wqp, Wkp, Wvp, Wqd, Wkd, Wvd):
    """Exact reference math in numpy (fallback for non-trivial masks)."""
    INF = 1000000.0

    def group(x, m, gs):
        n, l, d = x.shape
        xg = x.reshape(n, l // gs, gs, d).mean(axis=2)
        mg = m.reshape(n, l // gs, gs).any(axis=2)
        return xg, mg

    def heads(x):
        n, l, d = x.shape
        return x.reshape(n, l, H, d // H)

    pg, mp = group(protein, mask_prot, GS_P)
    dg, md = group(drug, mask_drug, GS_D)
    qp = heads(pg @ Wqp.T)
    kp = heads(pg @ Wkp.T)
    vp = heads(pg @ Wvp.T)
    qd = heads(dg @ Wqd.T)
    kd = heads(dg @ Wkd.T)
    vd = heads(dg @ Wvd.T)

    def alpha(logits, mr, mc):
        pair = mr[:, :, None, None] & mc[:, None, :, None]
        logits = np.where(pair, logits, logits - INF)
        m = logits.max(axis=2, keepdims=True)
        e = np.exp(logits - m)
        a = e / e.sum(axis=2, keepdims=True)
        return np.where(mr[:, :, None, None], a, 0.0)

    lpd = np.einsum("blhd,bkhd->blkh", qp, kd)
    ldp = np.einsum("blhd,bkhd->blkh", qd, kp)
    apd = alpha(lpd, mp, md)
    adp = alpha(ldp, md, mp)
    n = pg.shape[0]
    pe = np.einsum("blkh,bkhd->blhd", apd, vd).reshape(n, pg.shape[1], -1)
    de = np.einsum("blkh,bkhd->blhd", adp, vp).reshape(n, dg.shape[1], -1)
    mpf = mp.astype(pe.dtype)
    mdf = md.astype(de.dtype)
    pemb = (pe * mpf[:, :, None]).sum(axis=1) / mpf.sum(axis=-1)[:, None]
    demb = (de * mdf[:, :, None]).sum(axis=1) / mdf.sum(axis=-1)[:, None]
    return np.concatenate([pemb, demb], axis=1).astype(np.float32)


def _split_excess_waits(nc):
    """Split multi-sem waits into single-wait engine NOPs.

    TPB compute-instruction encodings carry exactly one sync-wait slot
    (NEURON_ISA_TPB_EVENTS); Tile sometimes assigns 2-3 waits to one
    instruction (psum slot tenancy transitions), which walrus rejects with
    "Too many sync wait commands". Since each engine dispatches its stream
    in order, prefixing the instruction with NOPs that each carry one of
    the excess waits is semantically identical.

    DMA instructions are different: their wait condition lives in the DGE
    descriptor and fires autonomously, so a NOP placed before them in the SP
    stream does NOT gate the transfer. For multi-wait DMAs we instead chain
    all waits through SP NOPs that bump a fresh "gate" semaphore, and give
    the descriptor a single gate>=k wait.
    """
    import concourse.mybir as mybir
    import bass_rust

    MULTI_OK = {"InstEventSemaphore"}

    def make_nop(engine):
        eng = {
            mybir.EngineType.PE: nc.tensor,
            mybir.EngineType.Activation: nc.scalar,
            mybir.EngineType.DVE: nc.vector,
            mybir.EngineType.Pool: nc.gpsimd,
            mybir.EngineType.SP: nc.sync,
        }[engine]
        bi = eng.nop(nofuse=True)
        inst = bi.ins if hasattr(bi, "ins") else bi
        # remove from wherever add_instruction appended it
        for bbw in nc.bb_map.values():
            lst = bbw.bb.instructions
            if lst and lst[-1] is inst:
                lst.pop()
                break
        return inst

    # pick a free semaphore id for the DMA gate
    used = set()
    for bbw in nc.bb_map.values():
        for inst in bbw.bb.instructions:
            si = getattr(inst, "sync_info", None)
            if si is None:
                continue
            for w in si.on_wait or []:
                used.add(w.id)
            for u in si.on_update or []:
                used.add(u.id)
    gate_id = max(used) + 1 if used else 100
    assert gate_id < 250, f"no free semaphore for DMA gate ({gate_id})"
    gate_count = 0

    n_split = 0
    for bbw in list(nc.bb_map.values()):
        bb = bbw.bb
        lst = bb.instructions
        idx = 0
        while idx < len(lst):
            inst = lst[idx]
            si = getattr(inst, "sync_info", None)
            if (
                si is not None
                and si.on_wait
                and len(si.on_wait) > 1
                and type(inst).__name__ not in MULTI_OK
            ):
                waits = list(si.on_wait)
                if type(inst).__name__ == "InstDMACopy":
                    # all waits go to SP nops; last nop bumps the gate;
                    # descriptor waits on the gate alone.
                    for w in waits:
                        nop = make_nop(mybir.EngineType.SP)
                        nop.sync_info = type(si)(on_wait=[w], on_update=[])
                        lst.insert(idx, nop)
                        idx += 1
                        n_split += 1
                    gate_count += 1
                    nop.sync_info = type(si)(
                        on_wait=[w],
                        on_update=[
                            bass_rust.SyncUpdate(
                                sync_type="semaphore",
                                id=gate_id,
                                ant_name=f"dma_gate_{gate_id}",
                                update_mode="sem-inc",
                                update_value=1,
                                update_reg=None,
                            )
                        ],
                    )
                    inst.sync_info = type(si)(
                        on_wait=[
                            bass_rust.SyncWait(
                                sync_type="semaphore",
                                id=gate_id,
                                ant_name=f"dma_gate_{gate_id}",
                                wait_mode="sem-ge-imm",
                                wait_value=gate_count,
                                wait_reg=None,
                            )
                        ],
                        on_update=si.on_update,
                    )
                else:
                    extra, keep = waits[:-1], waits[-1:]
                    for w in extra:
                        nop = make_nop(inst.engine)
                        nop.sync_info = type(si)(on_wait=[w], on_update=[])
                        lst.insert(idx, nop)
                        idx += 1
                        n_split += 1
                    inst.sync_info = type(si)(on_wait=keep, on_update=si.on_update)
            idx += 1
    return n_split


def _build_nc(stage_limit=99, dump_b=0, dump_stage=None):
    import concourse.bass as bass
    import concourse.mybir as mybir
    import concourse.tile as tile

    bf16 = mybir.dt.bfloat16
    f32 = mybir.dt.float32
    AF = mybir.ActivationFunctionType
    AX = mybir.AxisListType

    nc = bass.Bass()

    prot = nc.declare_dram_parameter("protein", [B, LP_FULL, HID], bf16, isOutput=False)
    drug = nc.declare_dram_parameter("drug", [B, LD_FULL, HID], bf16, isOutput=False)
    wnames = ["Wqp", "Wkp", "Wvp", "Wqd", "Wkd", "Wvd"]
    wdram = {
        w: nc.declare_dram_parameter(w, [HID, HID], bf16, isOutput=False)
        for w in wnames
    }
    wp_d = nc.declare_dram_parameter("wp", [B, LP], f32, isOutput=False)
    wd_d = nc.declare_dram_parameter("wd", [B, LD], f32, isOutput=False)
    gp_d = nc.declare_dram_parameter("Gp", [P, P // GS_P], bf16, isOutput=False)
    gd_d = nc.declare_dram_parameter("Gd", [P, P // GS_D], bf16, isOutput=False)
    out_d = nc.declare_dram_parameter("out", [B, 2 * HID], f32, isOutput=True)
    dbg_d = nc.declare_dram_parameter("dbg", [P, 4096], bf16, isOutput=True)

    KT = HID // P  # 4 contraction tiles over hidden dim

    with tile.TileContext(nc) as tc:
        with (
            tc.tile_pool(name="const", bufs=1) as cpool,
            tc.tile_pool(name="pt", bufs=2) as ptpool,
            tc.tile_pool(name="act", bufs=2) as apool,
            tc.tile_pool(name="pbig", bufs=4, space="PSUM") as pbig,
            tc.tile_pool(name="psmall", bufs=2, space="PSUM") as psmall,
            tc.tile_pool(name="pout", bufs=2, space="PSUM") as pout,
        ):
            # ---- constants: grouping matrices + 6 weight matrices ----
            gp_sb = cpool.tile([P, P // GS_P], bf16, tag="gp")
            nc.sync.dma_start(out=gp_sb, in_=gp_d[:, :])
            gd_sb = cpool.tile([P, P // GS_D], bf16, tag="gd")
            nc.sync.dma_start(out=gd_sb, in_=gd_d[:, :])
            w_sb = {}
            for w in wnames:
                t = cpool.tile([P, KT, HID], bf16, tag=f"w_{w}")
                for kt in range(KT):
                    nc.sync.dma_start(
                        out=t[:, kt, :], in_=wdram[w][kt * P : (kt + 1) * P, :]
                    )
                w_sb[w] = t

            out_stage = None
            if stage_limit >= 6:
                out_stage = cpool.tile(
                    [1, B, 2 * HID], f32, tag="out_stage", name="out_stage"
                )

            for b in range(B):
                # ---- load protein/drug tiles (natural [l, d] layout) ----
                pts = []
                for t in range(LP_FULL // P):
                    pt = ptpool.tile([P, HID], bf16, tag=f"pt{t}")
                    nc.sync.dma_start(out=pt, in_=prot[b, t * P : (t + 1) * P, :])
                    pts.append(pt)
                dts = []
                for t in range(LD_FULL // P):
                    dt = ptpool.tile([P, HID], bf16, tag=f"dt{t}")
                    nc.sync.dma_start(out=dt, in_=drug[b, t * P : (t + 1) * P, :])
                    dts.append(dt)
                # pooling weight vectors  [p, lt] layout
                wp_sb = apool.tile([P, LP // P], f32, tag="wp")
                nc.sync.dma_start(
                    out=wp_sb, in_=wp_d[b].rearrange("(t p) -> p t", p=P)
                )
                wd_sb = apool.tile([P, LD // P], f32, tag="wd")
                nc.sync.dma_start(
                    out=wd_sb, in_=wd_d[b].rearrange("(t p) -> p t", p=P)
                )

                # ---- grouping: pgT [d, lp] via matmul with protein as lhsT ----
                # pgT[d, 32t+g] = sum_l pt[t][l, d] * Gp[l, g]
                pgT = apool.tile([P, KT, LP], bf16, tag="pgT")
                for kt in range(KT):
                    ps = pbig.tile([P, LP], f32, tag="A")
                    for t in range(LP_FULL // P):
                        nc.tensor.matmul(
                            ps[:, t * 32 : (t + 1) * 32],
                            lhsT=pts[t][:, kt * P : (kt + 1) * P],
                            rhs=gp_sb,
                            start=True,
                            stop=True,
                        )
                    nc.scalar.copy(out=pgT[:, kt, :], in_=ps)
                # dgT [d, ld]: one psum tile [128, (kt, 128)]
                dgT = apool.tile([P, KT, LD], bf16, tag="dgT")
                ps = pbig.tile([P, LP], f32, tag="A")
                for kt in range(KT):
                    for t in range(LD_FULL // P):
                        nc.tensor.matmul(
                            ps[:, kt * LD + t * 64 : kt * LD + (t + 1) * 64],
                            lhsT=dts[t][:, kt * P : (kt + 1) * P],
                            rhs=gd_sb,
                            start=True,
                            stop=True,
                        )
                nc.vector.tensor_copy(
                    out=dgT.rearrange("p a b -> p (a b)"), in_=ps
                )
                if b == dump_b and (stage_limit == 1 or dump_stage == 1):
                    nc.sync.dma_start(
                        out=dbg_d[:, 0 : KT * LP],
                        in_=pgT.rearrange("p a b -> p (a b)"),
                    )
                    nc.sync.dma_start(
                        out=dbg_d[:, KT * LP : KT * LP + KT * LD],
                        in_=dgT.rearrange("p a b -> p (a b)"),
                    )
                if stage_limit < 2:
                    continue

                # ---- projections ----
                # qpT/kpT: [o, lp] layout = W.T-as-lhsT @ pgT
                def proj_T(wname, src, src_len, tag, evac):
                    dst = apool.tile([P, KT, src_len], bf16, tag=tag)
                    for mt in range(KT):  # output o-tiles
                        ps = pbig.tile([P, LP], f32, tag="A")
                        for kt in range(KT):
                            nc.tensor.matmul(
                                ps[:, :src_len],
                                lhsT=w_sb[wname][:, kt, mt * P : (mt + 1) * P],
                                rhs=src[:, kt, :],
                                start=(kt == 0),
                                stop=(kt == KT - 1),
                            )
                        evac(out=dst[:, mt, :], in_=ps[:, :src_len])
                    return dst

                qpT = proj_T("Wqp", pgT, LP, "qpT", nc.scalar.copy)
                kpT = proj_T("Wkp", pgT, LP, "kpT", nc.scalar.copy)
                qdT = proj_T("Wqd", dgT, LD, "qdT", nc.vector.tensor_copy)
                kdT = proj_T("Wkd", dgT, LD, "kdT", nc.vector.tensor_copy)

                # vp natural [lp, o]: lhsT = pgT block, rhs = Wvp.T
                vp = apool.tile([P, KT, HID], bf16, tag="vp")
                for mt in range(KT):  # lp-tiles
                    ps = pbig.tile([P, LP], f32, tag="A")
                    for kt in range(KT):
                        nc.tensor.matmul(
                            ps,
                            lhsT=pgT[:, kt, mt * P : (mt + 1) * P],
                            rhs=w_sb["Wvp"][:, kt, :],
                            start=(kt == 0),
                            stop=(kt == KT - 1),
                        )
                    nc.scalar.copy(out=vp[:, mt, :], in_=ps)
                # vd natural [ld, o]
                vd = apool.tile([P, HID], bf16, tag="vd")
                ps = pbig.tile([P, LP], f32, tag="A")
                for kt in range(KT):
                    nc.tensor.matmul(
                        ps,
                        lhsT=dgT[:, kt, :],
                        rhs=w_sb["Wvd"][:, kt, :],
                        start=(kt == 0),
                        stop=(kt == KT - 1),
                    )
                nc.vector.tensor_copy(out=vd, in_=ps)

                if b == dump_b and (stage_limit == 2 or dump_stage == 2):
                    nc.sync.dma_start(
                        out=dbg_d[:, 0 : KT * LP],
                        in_=qpT.rearrange("p a b -> p (a b)"),
                    )
                    nc.sync.dma_start(
                        out=dbg_d[:, KT * LP : 2 * KT * LP],
                        in_=vp.rearrange("p a b -> p (a b)"),
                    )
                if stage_limit < 2.1:
                    continue

                def head_slice(tens, h):
                    # [o, l]-layout tile [128, KT, len]: head h rows
                    return tens[64 * (h % 2) : 64 * (h % 2) + 64, h // 2, :]

                # ---- protein->drug attention: S_pd [lp, (h, ld)] ----
                P_pd = []  # per lp-tile: [128, H, LD] bf16
                rs_pd = apool.tile([P, LP // P, H], f32, tag="rs_pd")
                for lt in range(LP // P):
                    ptile = apool.tile([P, H, LD], bf16, tag=f"Ppd{lt}")
                    # even/odd heads use PE row-groups 0/64; concurrent
                    # row-group matmuls into the same PSUM bank fault the HW,
                    # so keep each parity in its own bank.
                    for par in range(2):
                        ps = pbig.tile([P, LP], f32, tag="A")
                        for hh in range(4):
                            h = 2 * hh + par
                            nc.tensor.matmul(
                                ps[:, hh * LD : (hh + 1) * LD],
                                lhsT=head_slice(qpT, h)[:, lt * P : (lt + 1) * P],
                                rhs=head_slice(kdT, h),
                                start=True,
                                stop=True,
                            )
                        nc.scalar.activation(
                            out=ptile[:, par : H : 2, :],
                            in_=ps,
                            func=AF.Exp,
                        )
                    if stage_limit >= 2.5:
                        nc.vector.reduce_sum(
                            out=rs_pd[:, lt, :], in_=ptile, axis=AX.X
                        )
                    P_pd.append(ptile)
                if b == dump_b and (stage_limit == 2.2 or dump_stage == 2.2):
                    nc.sync.dma_start(
                        out=dbg_d[:, 0 : H * LD],
                        in_=P_pd[0].rearrange("p a b -> p (a b)"),
                    )
                    nc.sync.dma_start(
                        out=dbg_d[:, H * LD : 2 * H * LD],
                        in_=P_pd[3].rearrange("p a b -> p (a b)"),
                    )
                if stage_limit < 2.8:
                    continue
                # u_pd[lt] = wp / rowsum  -> bf16 [128, lt, H]
                u_pd = apool.tile([P, LP // P, H], bf16, tag="u_pd")
                inv = apool.tile([P, LP // P, H], f32, tag="inv_pd")
                nc.vector.reciprocal(
                    out=inv.rearrange("p a b -> p (a b)"),
                    in_=rs_pd.rearrange("p a b -> p (a b)"),
                )
                for lt in range(LP // P):
                    nc.vector.tensor_scalar_mul(
                        u_pd[:, lt, :], inv[:, lt, :], wp_sb[:, lt : lt + 1]
                    )

                if b == dump_b and (stage_limit == 2.8 or dump_stage == 2.8):
                    u_f32 = apool.tile([P, LP // P, H], f32, tag="u_dump")
                    nc.vector.tensor_copy(
                        out=u_f32.rearrange("p a b -> p (a b)"),
                        in_=rs_pd.rearrange("p a b -> p (a b)"),
                    )
                    nc.sync.dma_start(
                        out=dbg_d[:, 0:64],
                        in_=u_pd.rearrange("p a b -> p (a b)"),
                    )
                    # rowsums as bf16 for dump
                    rs_bf = apool.tile([P, LP // P, H], bf16, tag="rs_dump")
                    nc.vector.tensor_copy(
                        out=rs_bf.rearrange("p a b -> p (a b)"),
                        in_=rs_pd.rearrange("p a b -> p (a b)"),
                    )
                    nc.sync.dma_start(
                        out=dbg_d[:, 64:128],
                        in_=rs_bf.rearrange("p a b -> p (a b)"),
                    )
                if stage_limit < 4:
                    continue

                # ---- drug->protein attention: S_dp [ld, (h, lp)] ----
                P_dp = apool.tile([P, H, LP], bf16, tag="Pdp")
                for h in range(H):
                    ps = pbig.tile([P, LP], f32, tag="A")
                    nc.tensor.matmul(
                        ps,
                        lhsT=head_slice(qdT, h),
                        rhs=head_slice(kpT, h),
                        start=True,
                        stop=True,
                    )
                    nc.scalar.activation(out=P_dp[:, h, :], in_=ps, func=AF.Exp)
                rs_dp = apool.tile([P, H], f32, tag="rs_dp")
                nc.vector.reduce_sum(out=rs_dp, in_=P_dp, axis=AX.X)
                u_dp = apool.tile([P, H], bf16, tag="u_dp")
                inv2 = apool.tile([P, H], f32, tag="inv_dp")
                nc.vector.reciprocal(out=inv2, in_=rs_dp)
                nc.vector.tensor_scalar_mul(u_dp, inv2, wd_sb[:, 0:1])

                if b == dump_b and (stage_limit == 4 or dump_stage == 4):
                    nc.sync.dma_start(
                        out=dbg_d[:, 0:4096],
                        in_=P_dp.rearrange("p a b -> p (a b)"),
                    )
                if stage_limit < 5:
                    continue

                # ---- c vectors ----
                # c_pdT [ld, h] = sum_lp P_pd[lp, h, ld] * u_pd[lp, h]
                ps_c = psmall.tile([P, KT, H], f32, tag="C")
                for h in range(H):
                    for lt in range(LP // P):
                        nc.tensor.matmul(
                            ps_c[:, 0, h : h + 1],
                            lhsT=P_pd[lt][:, h, :],
                            rhs=u_pd[:, lt, h : h + 1],
                            start=(lt == 0),
                            stop=(lt == LP // P - 1),
                        )
                c_pdT = apool.tile([P, H], bf16, tag="c_pdT")
                nc.vector.tensor_copy(out=c_pdT, in_=ps_c[:, 0, :])
                # c_dpT [lp-sub, (lt, h)] = sum_ld P_dp[ld, h, lp] * u_dp[ld, h]
                ps_c2 = psmall.tile([P, KT, H], f32, tag="C")
                for h in range(H):
                    for lt in range(LP // P):
                        nc.tensor.matmul(
                            ps_c2[:, lt, h : h + 1],
                            lhsT=P_dp[:, h, lt * P : (lt + 1) * P],
                            rhs=u_dp[:, h : h + 1],
                            start=True,
                            stop=True,
                        )
                c_dpT = apool.tile([P, KT, H], bf16, tag="c_dpT")
                nc.vector.tensor_copy(
                    out=c_dpT.rearrange("p a b -> p (a b)"),
                    in_=ps_c2.rearrange("p a b -> p (a b)"),
                )

                if b == dump_b and (stage_limit == 5 or dump_stage == 5):
                    nc.sync.dma_start(out=dbg_d[:, 0:H], in_=c_pdT)
                    nc.sync.dma_start(
                        out=dbg_d[:, H : H + KT * H],
                        in_=c_dpT.rearrange("p a b -> p (a b)"),
                    )
                if stage_limit < 6:
                    continue

                # ---- final embeddings ----
                ps_o1 = pout.tile([1, HID], f32, tag="O")
                for h in range(H):
                    nc.tensor.matmul(
                        ps_o1[:, h * DH : (h + 1) * DH],
                        lhsT=c_pdT[:, h : h + 1],
                        rhs=vd[:, h * DH : (h + 1) * DH],
                        start=True,
                        stop=True,
                    )
                ps_o2 = pout.tile([1, HID], f32, tag="O")
                for h in range(H):
                    for lt in range(LP // P):
                        nc.tensor.matmul(
                            ps_o2[:, h * DH : (h + 1) * DH],
                            lhsT=c_dpT[:, lt, h : h + 1],
                            rhs=vp[:, lt, h * DH : (h + 1) * DH],
                            start=(lt == 0),
                            stop=(lt == LP // P - 1),
                        )
                nc.scalar.copy(out=out_stage[:, b, 0:HID], in_=ps_o1)
                nc.scalar.copy(out=out_stage[:, b, HID : 2 * HID], in_=ps_o2)
                if b == dump_b and dump_stage == 6:
                    nc.sync.dma_start(out=dbg_d[:, 0:HID], in_=vd)
                    nc.sync.dma_start(
                        out=dbg_d[:, HID : HID + KT * HID],
                        in_=vp.rearrange("p a b -> p (a b)"),
                    )

            if out_stage is not None:
                nc.sync.dma_start(
                    out=out_d[:, :], in_=out_stage.rearrange("p a b -> p (a b)")
                )

    _split_excess_waits(nc)
    return nc


def _prep_in_maps(inputs):
    """Returns (in_maps, None) for the device path, or (None, fallback_out)."""
    protein = np.asarray(inputs["protein"], dtype=np.float32)
    drug = np.asarray(inputs["drug"], dtype=np.float32)
    mask_prot = np.asarray(inputs["mask_prot"]).astype(bool)
    mask_drug = np.asarray(inputs["mask_drug"]).astype(bool)
    Ws = {w: np.asarray(inputs[w], dtype=np.float32) for w in
          ["Wqp", "Wkp", "Wvp", "Wqd", "Wkd", "Wvd"]}

    import ml_dtypes

    bf = ml_dtypes.bfloat16

    # Grouped masks / pooling weights (general in the pooling path).
    mp = mask_prot.reshape(NB, LP, GS_P).any(axis=2)
    md = mask_drug.reshape(NB, LD, GS_D).any(axis=2)
    if not (mp.all() and md.all()):
        # Masked-out grouped positions change the softmax column masking —
        # handled exactly on the host (inputs per spec are all-ones).
        return None, _numpy_reference(
            protein, drug, mask_prot, mask_drug,
            Ws["Wqp"], Ws["Wkp"], Ws["Wvp"], Ws["Wqd"], Ws["Wkd"], Ws["Wvd"],
        )
    wp = (mp.astype(np.float32) / mp.sum(axis=1, keepdims=True)).astype(np.float32)
    wd = (md.astype(np.float32) / md.sum(axis=1, keepdims=True)).astype(np.float32)

    # Host-side layout prep (cheap): bf16 casts + weight transposes.
    prot_bf = protein.astype(bf)
    drug_bf = drug.astype(bf)
    wT = {w: np.ascontiguousarray(Ws[w].T).astype(bf) for w in Ws}
    gp = np.zeros((P, P // GS_P), dtype=bf)
    for g in range(P // GS_P):
        gp[GS_P * g : GS_P * (g + 1), g] = 1.0 / GS_P
    gd = np.zeros((P, P // GS_D), dtype=bf)
    for g in range(P // GS_D):
        gd[GS_D * g : GS_D * (g + 1), g] = 1.0 / GS_D

    in_maps = []
    for c in range(NCORES):
        sl = slice(c * B, (c + 1) * B)
        in_maps.append(
            {
                "protein": np.ascontiguousarray(prot_bf[sl]),
                "drug": np.ascontiguousarray(drug_bf[sl]),
                "wp": np.ascontiguousarray(wp[sl]),
                "wd": np.ascontiguousarray(wd[sl]),
                "Gp": gp,
                "Gd": gd,
                **{w: wT[w] for w in wT},
            }
        )
    return in_maps, None


def kernel(**inputs):
    in_maps, fallback = _prep_in_maps(inputs)
    if in_maps is None:
        return fallback

    if "nc" not in _CACHE:
        _CACHE["nc"] = _build_nc()
    nc = _CACHE["nc"]

    from concourse.bass_utils import run_bass_kernel_spmd

    res = run_bass_kernel_spmd(nc, in_maps, list(range(NCORES)))
    _CACHE["last_results"] = res
    out = np.concatenate([res.results[c]["out"] for c in range(NCORES)], axis=0)
    return out.astype(np.float32)


def run_traced(inputs):
    """Dev helper: traced HW run for profiling (returns BassKernelResults)."""
    in_maps, _ = _prep_in_maps(inputs)
    if in_maps is None:
        return None
    if "nc" not in _CACHE:
        _CACHE["nc"] = _build_nc()
    from concourse.bass_utils import run_bass_kernel_spmd

    return run_bass_kernel_spmd(_CACHE["nc"], in_maps, list(range(NCORES)), trace=True)


if __name__ == "__main__":
    rng = np.random.default_rng(0)
    inputs = {
        "protein": rng.standard_normal((NB, LP_FULL, HID), dtype=np.float32),
        "drug": rng.standard_normal((NB, LD_FULL, HID), dtype=np.float32),
        "mask_prot": np.ones((NB, LP_FULL), dtype=bool),
        "mask_drug": np.ones((NB, LD_FULL), dtype=bool),
    }
    for w in ["Wqp", "Wkp", "Wvp", "Wqd", "Wkd", "Wvd"]:
        inputs[w] = rng.standard_normal((HID, HID), dtype=np.float32) / np.sqrt(HID)
    out = kernel(**inputs)
    ref = _numpy_reference(
        inputs["protein"], inputs["drug"], inputs["mask_prot"], inputs["mask_drug"],
        inputs["Wqp"], inputs["Wkp"], inputs["Wvp"],
        inputs["Wqd"], inputs["Wkd"], inputs["Wvd"],
    )
    err = np.abs(out - ref).max() / np.abs(ref).max()
    print("rel err:", err)



# revision 6
# speedup vs baseline: 1.0480x; 1.0480x over previous
"""Trainium2 Bass kernel for nn_CrossLayer (protein/drug cross-attention).

Reference math (per batch n):
  pg = group_mean(protein, 4)   # (512, 512)
  dg = group_mean(drug, 2)      # (128, 512)
  q/k/v projections (8 heads, dh=64), cross logits, softmax over the
  "other" sequence, attention-weighted values, masked mean-pool over the
  own sequence, concat(prot_embed, drug_embed) -> (1024,)

Algebraic simplification: the pooled output never materializes the full
attention-output einsum; only per-head vectors c_h = P_h^T u_h (u = w /
rowsum) and a tiny c_h^T v matvec.

Performance structure (per core, 8 batches, data-parallel over N=64):
- All heavy matmuls run in fp8e4m3 with the DoubleRow perf mode (2 k-tiles
  per instruction, 0.5 cyc/col). Precision is restored by hi/lo splitting:
  x ~= x_hi + x_lo (both fp8, inputs pre-scaled so lo stays in e4m3 normal
  range) and 3-term products hi*hi + hi*lo + lo*hi, which costs 0.75x of a
  bf16 matmul while matching bf16 accuracy.
- Host pre-scales protein/drug by 4 and weights by 16; the 1/64 is folded
  into the (free) scale of the PSUM->SBUF evacuation.
- Logits/softmax run in bf16; rowsum_dp comes free from the activation
  accumulator; rowsum_pd is one DVE reduce.
- Evacuations are spread over Pool/DVE so the Tensor engine stays the
  bottleneck.
"""

import sys

import numpy as np

for _p in ("/opt/trn_rl_repo", "/root/.axon_site/_ro/trn_rl_repo"):
    if _p not in sys.path:
        sys.path.insert(0, _p)

HID = 512
H = 8
DH = 64
GS_P = 4
GS_D = 2
LP_FULL = 2048
LD_FULL = 256
LP = LP_FULL // GS_P  # 512 grouped protein length
LD = LD_FULL // GS_D  # 128 grouped drug length
NB = 64  # total batch
NCORES = 8
B = NB // NCORES  # 8 batches per core
P = 128  # partitions
KT = HID // P  # 4 contraction tiles over hidden dim
NTP = LP_FULL // P  # 16 protein l-tiles
NTD = LD_FULL // P  # 2 drug l-tiles

SP_SCALE = 4.0  # host scale on protein/drug (keeps fp8 lo terms normal)
SW_SCALE = 16.0  # host scale on weights
EVAC_SCALE = 1.0 / (SP_SCALE * SW_SCALE)

_CACHE = {}


def _numpy_reference(protein, drug, mask_prot, mask_drug, Wqp, Wkp, Wvp, Wqd, Wkd, Wvd):
    """Exact reference math in numpy (fallback for non-trivial masks)."""
    INF = 1000000.0

    def group(x, m, gs):
        n, l, d = x.shape
        xg = x.reshape(n, l // gs, gs, d).mean(axis=2)
        mg = m.reshape(n, l // gs, gs).any(axis=2)
        return xg, mg

    def heads(x):
        n, l, d = x.shape
        return x.reshape(n, l, H, d // H)

    pg, mp = group(protein, mask_prot, GS_P)
    dg, md = group(drug, mask_drug, GS_D)
    qp = heads(pg @ Wqp.T)
    kp = heads(pg @ Wkp.T)
    vp = heads(pg @ Wvp.T)
    qd = heads(dg @ Wqd.T)
    kd = heads(dg @ Wkd.T)
    vd = heads(dg @ Wvd.T)

    def alpha(logits, mr, mc):
        pair = mr[:, :, None, None] & mc[:, None, :, None]
        logits = np.where(pair, logits, logits - INF)
        m = logits.max(axis=2, keepdims=True)
        e = np.exp(logits - m)
        a = e / e.sum(axis=2, keepdims=True)
        return np.where(mr[:, :, None, None], a, 0.0)

    lpd = np.einsum("blhd,bkhd->blkh", qp, kd)
    ldp = np.einsum("blhd,bkhd->blkh", qd, kp)
    apd = alpha(lpd, mp, md)
    adp = alpha(ldp, md, mp)
    n = pg.shape[0]
    pe = np.einsum("blkh,bkhd->blhd", apd, vd).reshape(n, pg.shape[1], -1)
    de = np.einsum("blkh,bkhd->blhd", adp, vp).reshape(n, dg.shape[1], -1)
    mpf = mp.astype(pe.dtype)
    mdf = md.astype(de.dtype)
    pemb = (pe * mpf[:, :, None]).sum(axis=1) / mpf.sum(axis=-1)[:, None]
    demb = (de * mdf[:, :, None]).sum(axis=1) / mdf.sum(axis=-1)[:, None]
    return np.concatenate([pemb, demb], axis=1).astype(np.float32)


def _split_excess_waits(nc):
    """Split multi-sem waits into single-wait engine NOPs.

    TPB compute-instruction encodings carry exactly one sync-wait slot;
    Tile sometimes assigns 2-3 waits to one instruction, which walrus
    rejects. Since each engine dispatches its stream in order, prefixing
    the instruction with NOPs that each carry one of the excess waits is
    semantically identical.

    DMA instructions are different: their wait condition lives in the DGE
    descriptor and fires autonomously, so all waits are chained through SP
    NOPs that bump a fresh "gate" semaphore, and the descriptor gets a
    single gate>=k wait.
    """
    import concourse.mybir as mybir
    import bass_rust

    MULTI_OK = {"InstEventSemaphore"}

    def make_nop(engine):
        eng = {
            mybir.EngineType.PE: nc.tensor,
            mybir.EngineType.Activation: nc.scalar,
            mybir.EngineType.DVE: nc.vector,
            mybir.EngineType.Pool: nc.gpsimd,
            mybir.EngineType.SP: nc.sync,
        }[engine]
        bi = eng.nop(nofuse=True)
        inst = bi.ins if hasattr(bi, "ins") else bi
        for bbw in nc.bb_map.values():
            lst = bbw.bb.instructions
            if lst and lst[-1] is inst:
                lst.pop()
                break
        return inst

    used = set()
    for bbw in nc.bb_map.values():
        for inst in bbw.bb.instructions:
            si = getattr(inst, "sync_info", None)
            if si is None:
                continue
            for w in si.on_wait or []:
                used.add(w.id)
            for u in si.on_update or []:
                used.add(u.id)
    gate_id = max(used) + 1 if used else 100
    assert gate_id < 250, f"no free semaphore for DMA gate ({gate_id})"
    gate_count = 0

    n_split = 0
    for bbw in list(nc.bb_map.values()):
        bb = bbw.bb
        lst = bb.instructions
        idx = 0
        while idx < len(lst):
            inst = lst[idx]
            si = getattr(inst, "sync_info", None)
            if (
                si is not None
                and si.on_wait
                and len(si.on_wait) > 1
                and type(inst).__name__ not in MULTI_OK
            ):
                waits = list(si.on_wait)
                if type(inst).__name__ == "InstDMACopy":
                    for w in waits:
                        nop = make_nop(mybir.EngineType.SP)
                        nop.sync_info = type(si)(on_wait=[w], on_update=[])
                        lst.insert(idx, nop)
                        idx += 1
                        n_split += 1
                    gate_count += 1
                    nop.sync_info = type(si)(
                        on_wait=[w],
                        on_update=[
                            bass_rust.SyncUpdate(
                                sync_type="semaphore",
                                id=gate_id,
                                ant_name=f"dma_gate_{gate_id}",
                                update_mode="sem-inc",
                                update_value=1,
                                update_reg=None,
                            )
                        ],
                    )
                    inst.sync_info = type(si)(
                        on_wait=[
                            bass_rust.SyncWait(
                                sync_type="semaphore",
                                id=gate_id,
                                ant_name=f"dma_gate_{gate_id}",
                                wait_mode="sem-ge-imm",
                                wait_value=gate_count,
                                wait_reg=None,
                            )
                        ],
                        on_update=si.on_update,
                    )
                else:
                    extra, keep = waits[:-1], waits[-1:]
                    for w in extra:
                        nop = make_nop(inst.engine)
                        nop.sync_info = type(si)(on_wait=[w], on_update=[])
                        lst.insert(idx, nop)
                        idx += 1
                        n_split += 1
                    inst.sync_info = type(si)(on_wait=keep, on_update=si.on_update)
            idx += 1
    return n_split


def _build_nc():
    import concourse.bass as bass
    import concourse.mybir as mybir
    import concourse.tile as tile

    bf16 = mybir.dt.bfloat16
    f32 = mybir.dt.float32
    fp8 = mybir.dt.float8e4
    AF = mybir.ActivationFunctionType
    AX = mybir.AxisListType
    DR = mybir.MatmulPerfMode.DoubleRow

    nc = bass.Bass()

    # DRAM inputs. prot/drug carry interleaved (hi, lo) fp8 pairs per l-tile.
    prot = nc.declare_dram_parameter("protein", [B, NTP, P, 2, HID], fp8, isOutput=False)
    drug = nc.declare_dram_parameter("drug", [B, NTD, P, 2, HID], fp8, isOutput=False)
    wnames = ["Wqp", "Wkp", "Wvp", "Wqd", "Wkd", "Wvd"]
    wdram = {}
    for w in wnames:
        wdram[w + "h"] = nc.declare_dram_parameter(w + "h", [P, KT, HID], fp8, isOutput=False)
        wdram[w + "l"] = nc.declare_dram_parameter(w + "l", [P, KT, HID], fp8, isOutput=False)
    gp_d = nc.declare_dram_parameter("Gp", [P, 2, P // GS_P], fp8, isOutput=False)
    gd_d = nc.declare_dram_parameter("Gd", [P, 2, P // GS_D], fp8, isOutput=False)
    out_d = nc.declare_dram_parameter("out", [B, 2, H, DH], f32, isOutput=True)

    def hidx(h):
        # head position inside P_pd's (par, hh) free-dim ordering
        return (h % 2) * 4 + h // 2

    def hs(t, h, sl=slice(None)):
        # head slice of an [o, L]-layout tile [128, KT, L]
        return t[64 * (h % 2) : 64 * (h % 2) + 64, h // 2, sl]

    with tile.TileContext(nc) as tc:
        with (
            tc.tile_pool(name="const", bufs=1) as cpool,
            tc.tile_pool(name="pt", bufs=2) as ptpool,
            tc.tile_pool(name="act", bufs=2) as apool,
            tc.tile_pool(name="pproj", bufs=2, space="PSUM") as pproj,
            tc.tile_pool(name="ppd", bufs=2, space="PSUM") as ppd,
            tc.tile_pool(name="pdp", bufs=2, space="PSUM") as pdp,
        ):
            # ---- constants ----
            gp_sb = cpool.tile([P, 2, P // GS_P], fp8, tag="gp")
            nc.sync.dma_start(out=gp_sb, in_=gp_d[:, :, :])
            gd_sb = cpool.tile([P, 2, P // GS_D], fp8, tag="gd")
            nc.sync.dma_start(out=gd_sb, in_=gd_d[:, :, :])
            w_sb = {}
            for w in wdram:
                t = cpool.tile([P, KT, HID], fp8, tag=f"w_{w}")
                nc.sync.dma_start(out=t, in_=wdram[w][:, :, :])
                w_sb[w] = t
            sc_ev = cpool.tile([P, 1], f32, tag="sc_ev")
            nc.vector.memset(sc_ev, EVAC_SCALE)
            sc_up = cpool.tile([P, 1], f32, tag="sc_up")
            nc.vector.memset(sc_up, 1.0 / LP)
            sc_ud = cpool.tile([P, 1], f32, tag="sc_ud")
            nc.vector.memset(sc_ud, 1.0 / LD)

            for b in range(B):
                # ---- load protein/drug (hi, lo) tiles ----
                pts = []
                for t in range(NTP):
                    pt = ptpool.tile([P, 2, HID], fp8, tag=f"pt{t}")
                    nc.sync.dma_start(out=pt, in_=prot[b, t])
                    pts.append(pt)
                dts = []
                for t in range(NTD):
                    dt = ptpool.tile([P, 2, HID], fp8, tag=f"dt{t}")
                    nc.sync.dma_start(out=dt, in_=drug[b, t])
                    dts.append(dt)

                # ---- grouping (DoubleRow over (hi, lo)) ----
                # pgT[d, g] = sum_l (hi+lo)[l, d] * G[l, g], psum holds 4*pg
                pgh = apool.tile([P, KT, LP], fp8, tag="pgh")
                pgl = apool.tile([P, KT, LP], fp8, tag="pgl")
                for kt in range(KT):
                    ps = pproj.tile([P, LP], f32, tag="A")
                    for t in range(NTP):
                        nc.tensor.matmul(
                            ps[:, t * 32 : (t + 1) * 32],
                            lhsT=pts[t][:, :, kt * P : (kt + 1) * P],
                            rhs=gp_sb,
                            start=True,
                            stop=True,
                            perf_mode=DR,
                        )
                    nc.scalar.copy(out=pgh[:, kt, :], in_=ps)
                    nc.vector.tensor_sub(pgl[:, kt, :], ps, pgh[:, kt, :])
                dgh = apool.tile([P, KT, LD], fp8, tag="dgh")
                dgl = apool.tile([P, KT, LD], fp8, tag="dgl")
                ps = pproj.tile([P, LP], f32, tag="A")
                for kt in range(KT):
                    for t in range(NTD):
                        nc.tensor.matmul(
                            ps[:, kt * LD + t * 64 : kt * LD + (t + 1) * 64],
                            lhsT=dts[t][:, :, kt * P : (kt + 1) * P],
                            rhs=gd_sb,
                            start=True,
                            stop=True,
                            perf_mode=DR,
                        )
                nc.scalar.copy(
                    out=dgh.rearrange("p a b -> p (a b)"), in_=ps
                )
                nc.vector.tensor_sub(
                    dgl.rearrange("p a b -> p (a b)"), ps, dgh.rearrange("p a b -> p (a b)")
                )

                # ---- projections: 3-term fp8 DoubleRow ----
                def proj3(wname, srch, srcl, src_len, tag, dst_dt, evac):
                    """dst[o, l] layout [128, KT, src_len]; 6 DR matmuls per mt."""
                    dst = apool.tile([P, KT, src_len], dst_dt, tag=tag)
                    for mt in range(KT):
                        ps = pproj.tile([P, LP], f32, tag="A")
                        steps = []
                        for s in range(KT // 2):
                            ksl = slice(2 * s, 2 * s + 2)
                            msl = slice(mt * P, (mt + 1) * P)
                            steps.append((w_sb[wname + "h"][:, ksl, msl], srch[:, ksl, :]))
                            steps.append((w_sb[wname + "l"][:, ksl, msl], srch[:, ksl, :]))
                            steps.append((w_sb[wname + "h"][:, ksl, msl], srcl[:, ksl, :]))
                        for i, (lh, rh) in enumerate(steps):
                            nc.tensor.matmul(
                                ps[:, :src_len],
                                lhsT=lh,
                                rhs=rh,
                                start=(i == 0),
                                stop=(i == len(steps) - 1),
                                perf_mode=DR,
                            )
                        evac(dst[:, mt, :], ps[:, :src_len])
                    return dst

                def evac_dve(out, in_):
                    nc.vector.tensor_scalar_mul(out, in_, sc_ev)

                qpT = proj3("Wqp", pgh, pgl, LP, "qpT", bf16, evac_dve)
                kpT = proj3("Wkp", pgh, pgl, LP, "kpT", bf16, evac_dve)
                qdT = proj3("Wqd", dgh, dgl, LD, "qdT", bf16, evac_dve)
                kdT = proj3("Wkd", dgh, dgl, LD, "kdT", bf16, evac_dve)

                # vp natural [lp, o]: lhsT = pgT chunk, rhs = WvT tiles
                vp = apool.tile([P, KT, HID], bf16, tag="vp")
                for mt in range(KT):
                    ps = pproj.tile([P, LP], f32, tag="A")
                    steps = []
                    for s in range(KT // 2):
                        ksl = slice(2 * s, 2 * s + 2)
                        msl = slice(mt * P, (mt + 1) * P)
                        steps.append((pgh[:, ksl, msl], w_sb["Wvph"][:, ksl, :]))
                        steps.append((pgh[:, ksl, msl], w_sb["Wvpl"][:, ksl, :]))
                        steps.append((pgl[:, ksl, msl], w_sb["Wvph"][:, ksl, :]))
                    for i, (lh, rh) in enumerate(steps):
                        nc.tensor.matmul(
                            ps,
                            lhsT=lh,
                            rhs=rh,
                            start=(i == 0),
                            stop=(i == len(steps) - 1),
                            perf_mode=DR,
                        )
                    evac_dve(vp[:, mt, :], ps)
                vd = apool.tile([P, HID], bf16, tag="vd")
                ps = pproj.tile([P, LP], f32, tag="A")
                steps = []
                for s in range(KT // 2):
                    ksl = slice(2 * s, 2 * s + 2)
                    steps.append((dgh[:, ksl, :], w_sb["Wvdh"][:, ksl, :]))
                    steps.append((dgh[:, ksl, :], w_sb["Wvdl"][:, ksl, :]))
                    steps.append((dgl[:, ksl, :], w_sb["Wvdh"][:, ksl, :]))
                for i, (lh, rh) in enumerate(steps):
                    nc.tensor.matmul(
                        ps,
                        lhsT=lh,
                        rhs=rh,
                        start=(i == 0),
                        stop=(i == len(steps) - 1),
                        perf_mode=DR,
                    )
                evac_dve(vd, ps)

                # ---- protein->drug attention ----
                # P_pd [128, lt, (par, hh), ld]
                P_pd = apool.tile([P, LP // P, H, LD], bf16, tag="Ppd")
                for lt in range(LP // P):
                    ps = ppd.tile([P, H * LD], f32, tag="PD")
                    for par in range(2):
                        for hh in range(4):
                            h = 2 * hh + par
                            nc.tensor.matmul(
                                ps[:, par * 512 + hh * LD : par * 512 + (hh + 1) * LD],
                                lhsT=hs(qpT, h, slice(lt * P, (lt + 1) * P)),
                                rhs=hs(kdT, h),
                                start=True,
                                stop=True,
                            )
                    nc.scalar.activation(
                        out=P_pd[:, lt, :, :],
                        in_=ps,
                        func=AF.Exp,
                    )
                rs_pd = apool.tile([P, LP // P, H], f32, tag="rs_pd")
                nc.vector.reduce_sum(
                    out=rs_pd.rearrange("p a b -> p (a b)"),
                    in_=P_pd.rearrange("p a b c -> p (a b) c"),
                    axis=AX.X,
                )
                inv_pd = apool.tile([P, LP // P, H], f32, tag="inv_pd")
                nc.vector.reciprocal(
                    out=inv_pd.rearrange("p a b -> p (a b)"),
                    in_=rs_pd.rearrange("p a b -> p (a b)"),
                )
                u_pd = apool.tile([P, LP // P, H], bf16, tag="u_pd")
                nc.vector.tensor_scalar_mul(
                    u_pd.rearrange("p a b -> p (a b)"),
                    inv_pd.rearrange("p a b -> p (a b)"),
                    sc_up,
                )

                # ---- drug->protein attention ----
                P_dp = apool.tile([P, H, LP], bf16, tag="Pdp")
                rs_dp = apool.tile([P, H], f32, tag="rs_dp")
                for h in range(H):
                    ps = pdp.tile([P, LP], f32, tag="DP")
                    nc.tensor.matmul(
                        ps,
                        lhsT=hs(qdT, h),
                        rhs=hs(kpT, h),
                        start=True,
                        stop=True,
                    )
                    nc.scalar.activation(
                        out=P_dp[:, h, :],
                        in_=ps,
                        func=AF.Exp,
                        accum_out=rs_dp[:, h : h + 1],
                    )
                inv_dp = apool.tile([P, H], f32, tag="inv_dp")
                nc.vector.reciprocal(out=inv_dp, in_=rs_dp)
                u_dp = apool.tile([P, H], bf16, tag="u_dp")
                nc.vector.tensor_scalar_mul(u_dp, inv_dp, sc_ud)

                # ---- c vectors + final embeddings share one small psum tile ----
                # cols 0:8 = c_pd, 32:64 = c_dp, 96:112 (parts 0:64) = final
                ps_s = pdp.tile([P, LP], f32, tag="DP")
                ps_c = ps_s[:, 0:H]
                for h in range(H):
                    hx = hidx(h)
                    for lt in range(LP // P):
                        nc.tensor.matmul(
                            ps_c[:, h : h + 1],
                            lhsT=P_pd[:, lt, hx, :],
                            rhs=u_pd[:, lt, hx : hx + 1],
                            start=(lt == 0),
                            stop=(lt == LP // P - 1),
                        )
                c_pdT = apool.tile([P, H], bf16, tag="c_pdT")
                nc.vector.tensor_copy(out=c_pdT, in_=ps_c)
                ps_c2 = ps_s[:, 32:64]
                for h in range(H):
                    for lt in range(LP // P):
                        nc.tensor.matmul(
                            ps_c2[:, lt * H + h : lt * H + h + 1],
                            lhsT=P_dp[:, h, lt * P : (lt + 1) * P],
                            rhs=u_dp[:, h : h + 1],
                            start=True,
                            stop=True,
                        )
                c_dpT = apool.tile([P, LP // P, H], bf16, tag="c_dpT")
                nc.vector.tensor_copy(
                    out=c_dpT.rearrange("p a b -> p (a b)"),
                    in_=ps_c2,
                )

                # final: out[d, (x, h)] on 64 partitions
                ps_f = ps_s[0:DH, 96 : 96 + 2 * H]
                for h in range(H):
                    nc.tensor.matmul(
                        ps_f[:, h : h + 1],
                        lhsT=vd[:, h * DH : (h + 1) * DH],
                        rhs=c_pdT[:, h : h + 1],
                        start=True,
                        stop=True,
                    )
                for h in range(H):
                    for lt in range(LP // P):
                        nc.tensor.matmul(
                            ps_f[:, H + h : H + h + 1],
                            lhsT=vp[:, lt, h * DH : (h + 1) * DH],
                            rhs=c_dpT[:, lt, h : h + 1],
                            start=(lt == 0),
                            stop=(lt == LP // P - 1),
                        )
                f_sb = apool.tile([DH, 2 * H], f32, tag="f_sb")
                nc.vector.tensor_copy(out=f_sb, in_=ps_f)
                nc.sync.dma_start(
                    out=out_d[b].rearrange("x h d -> d (x h)"),
                    in_=f_sb,
                )

    _split_excess_waits(nc)
    return nc


def _prep_in_maps(inputs):
    """Returns (in_maps, None) for the device path, or (None, fallback_out)."""
    import ml_dtypes

    f8 = ml_dtypes.float8_e4m3fn

    protein = np.asarray(inputs["protein"], dtype=np.float32)
    drug = np.asarray(inputs["drug"], dtype=np.float32)
    mask_prot = np.asarray(inputs["mask_prot"]).astype(bool)
    mask_drug = np.asarray(inputs["mask_drug"]).astype(bool)
    Ws = {w: np.asarray(inputs[w], dtype=np.float32) for w in
          ["Wqp", "Wkp", "Wvp", "Wqd", "Wkd", "Wvd"]}

    mp = mask_prot.reshape(NB, LP, GS_P).any(axis=2)
    md = mask_drug.reshape(NB, LD, GS_D).any(axis=2)
    if not (mp.all() and md.all()):
        return None, _numpy_reference(
            protein, drug, mask_prot, mask_drug,
            Ws["Wqp"], Ws["Wkp"], Ws["Wvp"], Ws["Wqd"], Ws["Wkd"], Ws["Wvd"],
        )

    def split_hl(x):
        hi = x.astype(f8)
        lo = (x - hi.astype(np.float32)).astype(f8)
        return hi, lo

    # protein/drug: [NB, NT, 128, 2(hi/lo), 512] fp8, pre-scaled by 4
    def pack_seq(x, nt):
        xs = (x * SP_SCALE).reshape(NB, nt, P, HID)
        hi, lo = split_hl(xs)
        out = np.empty((NB, nt, P, 2, HID), dtype=f8)
        out[:, :, :, 0, :] = hi
        out[:, :, :, 1, :] = lo
        return out

    prot_hl = pack_seq(protein, NTP)
    drug_hl = pack_seq(drug, NTD)

    # weights: W.T * 16 -> [128, KT, 512] hi/lo fp8
    wmaps = {}
    for w, Wv in Ws.items():
        wt = np.ascontiguousarray(Wv.T * SW_SCALE).reshape(KT, P, HID)
        wt = np.transpose(wt, (1, 0, 2))  # [p, kt, o]
        hi, lo = split_hl(wt.astype(np.float32))
        wmaps[w + "h"] = np.ascontiguousarray(hi)
        wmaps[w + "l"] = np.ascontiguousarray(lo)

    gp = np.zeros((P, 2, P // GS_P), dtype=f8)
    for g in range(P // GS_P):
        gp[GS_P * g : GS_P * (g + 1), :, g] = 1.0 / GS_P
    gd = np.zeros((P, 2, P // GS_D), dtype=f8)
    for g in range(P // GS_D):
        gd[GS_D * g : GS_D * (g + 1), :, g] = 1.0 / GS_D

    in_maps = []
    for c in range(NCORES):
        sl = slice(c * B, (c + 1) * B)
        in_maps.append(
            {
                "protein": np.ascontiguousarray(prot_hl[sl]),
                "drug": np.ascontiguousarray(drug_hl[sl]),
                "Gp": gp,
                "Gd": gd,
                **wmaps,
            }
        )
    return in_maps, None


def kernel(**inputs):
    in_maps, fallback = _prep_in_maps(inputs)
    if in_maps is None:
        return fallback

    if "nc" not in _CACHE:
        _CACHE["nc"] = _build_nc()
    nc = _CACHE["nc"]

    from concourse.bass_utils import run_bass_kernel_spmd

    res = run_bass_kernel_spmd(nc, in_maps, list(range(NCORES)))
    _CACHE["last_results"] = res
    out = np.concatenate(
        [res.results[c]["out"].reshape(B, 2 * HID) for c in range(NCORES)], axis=0
    )
    return out.astype(np.float32)


def run_traced(inputs):
    """Dev helper: traced HW run for profiling (returns BassKernelResults)."""
    in_maps, _ = _prep_in_maps(inputs)
    if in_maps is None:
        return None
    if "nc" not in _CACHE:
        _CACHE["nc"] = _build_nc()
    from concourse.bass_utils import run_bass_kernel_spmd

    return run_bass_kernel_spmd(_CACHE["nc"], in_maps, list(range(NCORES)), trace=True)


if __name__ == "__main__":
    rng = np.random.default_rng(0)
    inputs = {
        "protein": rng.standard_normal((NB, LP_FULL, HID), dtype=np.float32),
        "drug": rng.standard_normal((NB, LD_FULL, HID), dtype=np.float32),
        "mask_prot": np.ones((NB, LP_FULL), dtype=bool),
        "mask_drug": np.ones((NB, LD_FULL), dtype=bool),
    }
    for w in ["Wqp", "Wkp", "Wvp", "Wqd", "Wkd", "Wvd"]:
        inputs[w] = rng.standard_normal((HID, HID), dtype=np.float32) / np.sqrt(HID)
    out = kernel(**inputs)
    ref = _numpy_reference(
        inputs["protein"], inputs["drug"], inputs["mask_prot"], inputs["mask_drug"],
        inputs["Wqp"], inputs["Wkp"], inputs["Wvp"],
        inputs["Wqd"], inputs["Wkd"], inputs["Wvd"],
    )
    err = np.abs(out - ref).max() / np.abs(ref).max()
    print("rel err:", err)
